# revision 1
# baseline (speedup 1.0000x reference)
"""Multi-head causal attention (b=4, t=2048, k=1024, h=16) on 8 Trainium2 cores.

Sharding: core c = (batch b=c//2, head-group g=c%2). Each core computes one
batch x 8 heads; the two half-head partial outputs per batch are summed on
host.

Per-core kernel (all tensors kept transposed so no on-chip transposes):
  A) QT = WqT.T @ xT, KT = WkT.T @ xT    [dq, t]   (bf16 in, fp32 psum)
     V  = xT.T @ WvT                      [t, dv]
  B) per head: ST[k,q] = KT_tile.T @ QT (f32r), exp on ACT, causal chunking;
     PV: OT_aug[65,q] += [V|1].T-style matmul accumulating over k tiles,
     row 64 = softmax denominator; normalize via DVE reciprocal + K=1
     broadcast matmul.
  C) out[t,o] = OT_tile.T @ WoT  (f32r)
"""
import sys

sys.path.insert(0, "/opt/trn_rl_repo")

import numpy as np
import ml_dtypes

import concourse.bass as bass
import concourse.mybir as mybir
import concourse.tile as tile
from concourse import bacc
from concourse.bass_utils import run_bass_kernel_spmd
from concourse.masks import make_upper_triangular

F32 = mybir.dt.float32
F32R = mybir.dt.float32r
BF16 = mybir.dt.bfloat16
EXP = mybir.ActivationFunctionType.Exp

B, T, KD, NH, HS = 4, 2048, 1024, 16, 64
NCORES = 8


def build_nc(t=T, dl=512, hl=8, kd=KD):
    """One core's program: x.T [kd,t], per-group weights, partial out [t,kd]."""
    nk = kd // 128       # contraction tiles for projections
    mt = t // 128        # t tiles (also k-position tiles in attention)
    dt = dl // 128       # local-dim tiles
    nqc = t // 512       # q chunks
    scale = 1.0 / float(np.sqrt(kd))

    nc = bacc.Bacc("TRN2", target_bir_lowering=False, debug=False, num_devices=NCORES)
    xt_d = nc.dram_tensor("xt", [kd, t], BF16, kind="ExternalInput")
    wq_d = nc.dram_tensor("wq", [kd, dl], BF16, kind="ExternalInput")
    wk_d = nc.dram_tensor("wk", [kd, dl], BF16, kind="ExternalInput")
    wv_d = nc.dram_tensor("wv", [kd, dl], BF16, kind="ExternalInput")
    wo_d = nc.dram_tensor("wo", [dl, kd], F32R, kind="ExternalInput")
    out_d = nc.dram_tensor("out", [t, kd], F32, kind="ExternalOutput")

    with tile.TileContext(nc) as tc:
        with (
            tc.tile_pool(name="persist", bufs=1) as pp,
            tc.tile_pool(name="misc", bufs=1) as mp,
        ):
            qt_s = pp.tile([128, dt, t], F32R)
            kt_s = pp.tile([128, dt, t], F32R)
            v_s = pp.tile([128, mt, hl, 65], F32R)
            mask_t = mp.tile([128, 128], F32)
            ones_t = mp.tile([1, 64], F32R)
            ones128 = mp.tile([128, 128], F32)
            make_upper_triangular(nc, mask_t[:, :], val=1.0, diag=True)
            nc.vector.memset(ones128[:, :], 1.0)
            nc.scalar.copy(ones_t[:, :], ones128[0:1, 0:64])
            nc.scalar.copy(
                v_s[:, :, :, 64],
                ones128[:, 0 : mt * hl].rearrange("p (m h) -> p m h", m=mt),
            )

            # ---------------- Phase A: projections ----------------
            with (
                tc.tile_pool(name="pa", bufs=1) as pa,
                tc.tile_pool(name="paps", bufs=8, space="PSUM") as paps,
            ):
                xt_s = pa.tile([128, nk, t], BF16)
                wq_s = pa.tile([128, nk, dl], BF16)
                wk_s = pa.tile([128, nk, dl], BF16)
                wv_s = pa.tile([128, nk, dl], BF16)
                nc.sync.dma_start(
                    wv_s[:, :, :], wv_d[:, :].rearrange("(n p) d -> p n d", p=128)
                )
                xt_r = xt_d[:, :].rearrange("(n p) t -> p n t", p=128)
                for k in range(nk):
                    nc.sync.dma_start(xt_s[:, k, :], xt_r[:, k, :])
                nc.sync.dma_start(
                    wq_s[:, :, :], wq_d[:, :].rearrange("(n p) d -> p n d", p=128)
                )
                nc.sync.dma_start(
                    wk_s[:, :, :], wk_d[:, :].rearrange("(n p) d -> p n d", p=128)
                )

                for m in range(mt):
                    psv = paps.tile([128, dl], F32, name=f"psv{m}", tag="proj")
                    for k in range(nk):
                        nc.tensor.matmul(
                            psv[:, :],
                            xt_s[:, k, 128 * m : 128 * m + 128],
                            wv_s[:, k, :],
                            start=(k == 0),
                            stop=(k == nk - 1),
                        )
                    dstv = v_s[:, m, :, 0:64]
                    srcv = psv[:, :].rearrange("p (h d) -> p h d", h=hl)
                    if m % 2 == 0:
                        nc.scalar.copy(dstv, srcv)
                    else:
                        nc.vector.tensor_copy(dstv, srcv)

                for m in range(dt):
                    for w_s, o_s, pfx in ((wq_s, qt_s, "q"), (wk_s, kt_s, "k")):
                        ps = [
                            paps.tile(
                                [128, 512], F32, name=f"ps{pfx}{m}_{n}", tag="proj"
                            )
                            for n in range(t // 512)
                        ]
                        for k in range(nk):
                            for n in range(t // 512):
                                nc.tensor.matmul(
                                    ps[n][:, :],
                                    w_s[:, k, 128 * m : 128 * m + 128],
                                    xt_s[:, k, 512 * n : 512 * n + 512],
                                    start=(k == 0),
                                    stop=(k == nk - 1),
                                )
                        for n in range(t // 512):
                            if n % 2 == 0:
                                nc.scalar.copy(
                                    o_s[:, m, 512 * n : 512 * n + 512], ps[n][:, :]
                                )
                            else:
                                nc.vector.tensor_copy(
                                    o_s[:, m, 512 * n : 512 * n + 512], ps[n][:, :]
                                )

            # ---------------- Phase B: attention ----------------
            with tc.tile_pool(name="pb", bufs=1) as pb:
                ot_s = pb.tile([128, dt, t], F32R)
                with (
                    tc.tile_pool(name="pbe", bufs=4) as pbe,
                    tc.tile_pool(name="pbm", bufs=2) as pbm,
                    tc.tile_pool(name="stps", bufs=3, space="PSUM") as stps,
                    tc.tile_pool(name="otps", bufs=4, space="PSUM") as otps,
                    tc.tile_pool(name="bcps", bufs=1, space="PSUM") as bcps,
                ):
                    for h in range(hl):
                        mh, ph = h // 2, 64 * (h % 2)
                        otp = [
                            otps.tile([128, 512], F32, name=f"otp{h}_{qc}", tag="ot")
                            for qc in range(nqc)
                        ]
                        for ki in range(mt):
                            q0 = 128 * ki
                            for qc in range(q0 // 512, nqc):
                                off = max(q0, 512 * qc)
                                w = 512 * (qc + 1) - off
                                stp = stps.tile(
                                    [128, 512], F32, name=f"st{h}_{ki}_{qc}", tag="st"
                                )
                                nc.tensor.matmul(
                                    stp[:, :w],
                                    kt_s[ph : ph + 64, mh, q0 : q0 + 128],
                                    qt_s[ph : ph + 64, mh, off : off + w],
                                    start=True,
                                    stop=True,
                                )
                                ex = pbe.tile(
                                    [128, 512], F32R, name=f"ex{h}_{ki}_{qc}", tag="exp"
                                )
                                nc.scalar.activation(
                                    ex[:, :w], stp[:, :w], EXP, scale=scale
                                )
                                if off == q0:
                                    nc.vector.tensor_mul(
                                        ex[:, 0:128], ex[:, 0:128], mask_t[:, :].bitcast(F32R)
                                    )
                                co = off - 512 * qc
                                nc.tensor.matmul(
                                    otp[qc][0:65, co : co + w],
                                    v_s[:, ki, h, :],
                                    ex[:, :w],
                                    start=(ki == 0),
                                    stop=(ki == 4 * qc + 3),
                                )
                        for qc in range(nqc):
                            rec = pbm.tile([1, 512], F32R, name=f"rc{h}_{qc}", tag="rec")
                            with nc.allow_low_precision(reason="softmax denom f32r"):
                                nc.vector.reciprocal(rec[:, :], otp[qc][64:65, :])
                            bc = bcps.tile([128, 512], F32, name=f"bc{h}_{qc}", tag="bc")
                            nc.tensor.matmul(
                                bc[0:64, :],
                                ones_t[:, :],
                                rec[:, :],
                                start=True,
                                stop=True,
                            )
                            cols = slice(512 * qc, 512 * qc + 512)
                            if ph == 0:
                                dst = ot_s[0:64, mh, cols]
                                nc.scalar.copy(dst, otp[qc][0:64, :])
                                nc.vector.tensor_mul(dst, dst, bc[0:64, :].bitcast(F32R))
                            else:
                                sc = pbm.tile(
                                    [64, 512], F32R, name=f"sc{h}_{qc}", tag="scr"
                                )
                                nc.scalar.copy(sc[:, :], otp[qc][0:64, :])
                                nc.vector.tensor_mul(sc[:, :], sc[:, :], bc[0:64, :].bitcast(F32R))
                                nc.sync.dma_start(ot_s[64:128, mh, cols], sc[:, :])

                # ---------------- Phase C: output projection ----------------
                with (
                    tc.tile_pool(name="pc", bufs=1) as pc,
                    tc.tile_pool(name="pco", bufs=2) as pco,
                    tc.tile_pool(name="pcps", bufs=8, space="PSUM") as pcps,
                ):
                    wo_s = pc.tile([128, dt, kd], F32R)
                    nc.sync.dma_start(
                        wo_s[:, :, :], wo_d[:, :].rearrange("(n p) o -> p n o", p=128)
                    )
                    for m in range(mt):
                        pso = [
                            pcps.tile([128, 512], F32, name=f"pso{m}_{n}", tag="pc")
                            for n in range(kd // 512)
                        ]
                        for k in range(dt):
                            for n in range(kd // 512):
                                nc.tensor.matmul(
                                    pso[n][:, :],
                                    ot_s[:, k, 128 * m : 128 * m + 128],
                                    wo_s[:, k, 512 * n : 512 * n + 512],
                                    start=(k == 0),
                                    stop=(k == dt - 1),
                                )
                        ob = pco.tile([128, kd], F32, name=f"ob{m}", tag="ob")
                        for n in range(kd // 512):
                            nc.scalar.copy(ob[:, 512 * n : 512 * n + 512], pso[n][:, :])
                        nc.sync.dma_start(out_d[128 * m : 128 * m + 128, :], ob[:, :])

    nc.finalize()
    return nc


_NC_CACHE = {}


def _get_nc(key=(T, 512, 8, KD)):
    if key not in _NC_CACHE:
        _NC_CACHE[key] = build_nc(*key)
    return _NC_CACHE[key]


def make_in_maps(x, Wq, Wk, Wv, Wo, dl=512):
    in_maps = []
    for c in range(NCORES):
        b, g = c // 2, c % 2
        rows = slice(dl * g, dl * (g + 1))
        in_maps.append(
            {
                "xt": np.ascontiguousarray(x[b].T).astype(ml_dtypes.bfloat16),
                "wq": np.ascontiguousarray(Wq[rows, :].T).astype(ml_dtypes.bfloat16),
                "wk": np.ascontiguousarray(Wk[rows, :].T).astype(ml_dtypes.bfloat16),
                "wv": np.ascontiguousarray(Wv[rows, :].T).astype(ml_dtypes.bfloat16),
                "wo": np.ascontiguousarray(Wo[:, rows].T).astype(np.float32),
            }
        )
    return in_maps


def run_spmd(x, Wq, Wk, Wv, Wo, trace=False):
    nc = _get_nc()
    in_maps = make_in_maps(x, Wq, Wk, Wv, Wo)
    res = run_bass_kernel_spmd(nc, in_maps, list(range(NCORES)), trace=trace)
    outs = [res.results[c]["out"] for c in range(NCORES)]
    final = np.stack([outs[2 * b] + outs[2 * b + 1] for b in range(B)])
    return final.astype(np.float32), res


def kernel(x, Wq, Wk, Wv, Wo):
    x = np.asarray(x, dtype=np.float32)
    Wq = np.asarray(Wq, dtype=np.float32)
    Wk = np.asarray(Wk, dtype=np.float32)
    Wv = np.asarray(Wv, dtype=np.float32)
    Wo = np.asarray(Wo, dtype=np.float32)
    out, _ = run_spmd(x, Wq, Wk, Wv, Wo)
    return out



# revision 8
# speedup vs baseline: 1.3967x; 1.3967x over previous
"""Multi-head causal attention (b=4, t=2048, k=1024, h=16) on 8 Trainium2 cores.

Sharding: core c = (batch b=c//2, head-group g=c%2). Each core computes one
batch x 8 heads; the two half-head partial outputs per batch are summed on
host.

Per-core kernel, v4: the tensor engine's DVFS only reaches full clock under
continuous work, so projection / output-projection matmuls are interleaved
into the attention stream as filler:
  - prologue: Q/K projections for head-pair 0 + V m-tiles 0-3
  - attention pair p (2 heads) runs with Q/K projections of pair p+1 (or V
    tiles, or phase-C output-projection tiles for the last pair) emitted
    between attention units, so the PE never idles while ACT runs exp.
  - per attention unit (ki, 512-col q chunk): ST matmul (bf16) into a
    1-bank psum, exp on ACT (bf16 out), diag-mask on DVE, PV accumulate
    [V|1].T @ ex (row 64 = softmax denom), normalize via DVE
    reciprocal_approx_fast + gpsimd partition_broadcast + DVE mul.
  PSUM budget: proj(2) + st(2) + otp(4) = 8 banks; phase C borrows the
  proj tag's banks during the last pair.
"""
import sys

sys.path.insert(0, "/opt/trn_rl_repo")

import numpy as np
import ml_dtypes

import concourse.bass as bass
import concourse.mybir as mybir
import concourse.tile as tile
from concourse import bacc
from concourse.bass_utils import run_bass_kernel_spmd
from concourse.masks import make_upper_triangular

F32 = mybir.dt.float32
BF16 = mybir.dt.bfloat16
EXP = mybir.ActivationFunctionType.Exp

B, T, KD, NH, HS = 4, 2048, 1024, 16, 64
NCORES = 8


def build_nc(t=T, dl=512, hl=8, kd=KD):
    nk = kd // 128       # contraction tiles for projections
    mt = t // 128        # t tiles (also k-position tiles in attention)
    dt = dl // 128       # local-dim tiles (head pairs)
    nqc = t // 512       # q chunks
    scale = 1.0 / float(np.sqrt(kd))

    nc = bacc.Bacc("TRN2", target_bir_lowering=False, debug=False, num_devices=NCORES)
    xt_d = nc.dram_tensor("xt", [kd, t], BF16, kind="ExternalInput")
    wq_d = nc.dram_tensor("wq", [kd, dl], BF16, kind="ExternalInput")
    wk_d = nc.dram_tensor("wk", [kd, dl], BF16, kind="ExternalInput")
    wv_d = nc.dram_tensor("wv", [kd, dl], BF16, kind="ExternalInput")
    wo_d = nc.dram_tensor("wo", [dl, kd], BF16, kind="ExternalInput")
    out_d = nc.dram_tensor("out", [t, kd], F32, kind="ExternalOutput")

    with tile.TileContext(nc) as tc:
        with (
            tc.tile_pool(name="persist", bufs=1) as pp,
            tc.tile_pool(name="misc", bufs=1) as mp,
            tc.tile_pool(name="pbe", bufs=4) as pbe,
            tc.tile_pool(name="pbm", bufs=4) as pbm,
            tc.tile_pool(name="pco", bufs=2) as pco,
            tc.tile_pool(name="psum", bufs=1, space="PSUM") as psp,
        ):
            qt_s = pp.tile([128, dt, t], BF16)
            kt_s = pp.tile([128, dt, t], BF16)
            v_s = pp.tile([128, mt, hl, 65], BF16)
            ot_s = pp.tile([128, dt, t], BF16)
            xt_s = pp.tile([128, nk, t], BF16)
            wq_s = pp.tile([128, nk, dl], BF16)
            wk_s = pp.tile([128, nk, dl], BF16)
            wv_s = pp.tile([128, nk, dl], BF16)
            wo_s = pp.tile([128, dt, kd], BF16)
            mask_f = mp.tile([128, 128], F32)
            mask_t = mp.tile([128, 128], BF16)
            make_upper_triangular(nc, mask_f[:, :], val=1.0, diag=True)
            nc.vector.tensor_copy(mask_t[:, :], mask_f[:, :])
            nc.vector.memset(v_s[:, :, :, 64], 1.0)

            # --------------- input DMA (fine-grained for fast start) -------
            wq_r = wq_d[:, :].rearrange("(n p) d -> p n d", p=128)
            wk_r = wk_d[:, :].rearrange("(n p) d -> p n d", p=128)
            xt_r = xt_d[:, :].rearrange("(n p) t -> p n t", p=128)
            for k in range(nk):
                nc.scalar.dma_start(wq_s[:, k, :], wq_r[:, k, :])
            for k in range(nk):
                nc.sync.dma_start(xt_s[:, k, 0:512], xt_r[:, k, 0:512])
            for k in range(nk):
                nc.scalar.dma_start(wk_s[:, k, :], wk_r[:, k, :])
            for n in range(1, t // 512):
                nc.sync.dma_start(
                    xt_s[:, :, 512 * n : 512 * n + 512],
                    xt_r[:, :, 512 * n : 512 * n + 512],
                )
            nc.scalar.dma_start(
                wv_s[:, :, :], wv_d[:, :].rearrange("(n p) d -> p n d", p=128)
            )
            nc.scalar.dma_start(
                wo_s[:, :, :], wo_d[:, :].rearrange("(n p) o -> p n o", p=128)
            )

            # --------------- filler emitters (projections / phase C) ------
            cnt = [0]

            def emit_qk(w_s, o_s, pair, n, on_act=False):
                """One Q or K projection psum group: rows 128*pair, cols n."""
                cols = slice(512 * n, 512 * n + 512)
                ps = psp.tile(
                    [128, 512], F32, name=f"pj{cnt[0]}", tag="proj", bufs=2
                )
                cnt[0] += 1
                for k in range(nk):
                    nc.tensor.matmul(
                        ps[:, :],
                        w_s[:, k, 128 * pair : 128 * pair + 128],
                        xt_s[:, k, cols],
                        start=(k == 0),
                        stop=(k == nk - 1),
                    )
                if on_act:
                    nc.scalar.copy(o_s[:, pair, cols], ps[:, :])
                else:
                    nc.vector.tensor_copy(o_s[:, pair, cols], ps[:, :])

            def emit_v(m, on_act=False):
                """V projection for t-tile m (all 8 heads at once)."""
                ps = psp.tile(
                    [128, 512], F32, name=f"pv{cnt[0]}", tag="proj", bufs=2
                )
                cnt[0] += 1
                for k in range(nk):
                    nc.tensor.matmul(
                        ps[:, :],
                        xt_s[:, k, 128 * m : 128 * m + 128],
                        wv_s[:, k, :],
                        start=(k == 0),
                        stop=(k == nk - 1),
                    )
                src = ps[:, :].rearrange("p (h d) -> p h d", h=hl)
                if on_act:
                    nc.scalar.copy(v_s[:, m, :, 0:64], src)
                else:
                    nc.vector.tensor_copy(v_s[:, m, :, 0:64], src)

            def emit_c(m):
                """Phase C output-projection for t-tile m (borrows proj banks)."""
                ob = pco.tile([128, kd], F32, name=f"ob{m}", tag="ob")
                for c in range(kd // 512):
                    ps = psp.tile(
                        [128, 512], F32, name=f"pc{cnt[0]}", tag="proj", bufs=2
                    )
                    cnt[0] += 1
                    for k in range(dt):
                        nc.tensor.matmul(
                            ps[:, :],
                            ot_s[:, k, 128 * m : 128 * m + 128],
                            wo_s[:, k, 512 * c : 512 * c + 512],
                            start=(k == 0),
                            stop=(k == dt - 1),
                        )
                    nc.vector.tensor_copy(ob[:, 512 * c : 512 * c + 512], ps[:, :])
                nc.sync.dma_start(out_d[128 * m : 128 * m + 128, :], ob[:, :])

            # --------------- prologue: QK(pair0) + V(0..3) -----------------
            for n in range(4):
                emit_qk(wq_s, qt_s, 0, n, on_act=True)
                emit_qk(wk_s, kt_s, 0, n, on_act=True)
            for m in range(4):
                emit_v(m, on_act=True)

            # --------------- fused attention + filler ----------------------
            otp = [None] * nqc

            def emit_pv(h, ki, qc, a, b, ex):
                mh, ph = h // 2, 64 * (h % 2)
                nc.tensor.matmul(
                    otp[qc][0:65, a - 512 * qc : b - 512 * qc],
                    v_s[:, ki, h, :],
                    ex[:, 0 : b - a],
                    start=(ki == 0),
                    stop=(ki == 4 * qc + 3),
                )
                if ki == 4 * qc + 3:
                    den = pbm.tile([1, 512], F32, name=f"dn{h}_{qc}", tag="den")
                    nc.vector.tensor_copy(den[:, :], otp[qc][64:65, :])
                    rec = pbm.tile([1, 512], F32, name=f"rc{h}_{qc}", tag="rec")
                    nc.vector.reciprocal_approx_fast(rec[:, :], den[:, :])
                    bc = pbm.tile([64, 512], F32, name=f"bc{h}_{qc}", tag="bc")
                    nc.gpsimd.partition_broadcast(bc[:, :], rec[0:1, :])
                    cols = slice(512 * qc, 512 * qc + 512)
                    if ph == 0:
                        nc.vector.tensor_mul(
                            ot_s[0:64, mh, cols], otp[qc][0:64, :], bc[:, :]
                        )
                    else:
                        sc = pbm.tile([64, 512], BF16, name=f"sc{h}_{qc}", tag="sc")
                        nc.vector.tensor_mul(sc[:, :], otp[qc][0:64, :], bc[:, :])
                        nc.sync.dma_start(ot_s[64:128, mh, cols], sc[:, :])
                    return qc
                return None

            for p in range(dt):
                # filler work for this pair's attention span: `front` items
                # are emitted one-per-unit from the start (ordering-critical
                # V tiles); `spread` items are paced evenly across the pair.
                front = []
                spread = []
                if p == 0:
                    front += [(lambda m=m: emit_v(m)) for m in range(4, mt)]
                if p < dt - 1:
                    for n in range(4):
                        spread.append(
                            lambda n=n, p=p: emit_qk(wq_s, qt_s, p + 1, n)
                        )
                        spread.append(
                            lambda n=n, p=p: emit_qk(wk_s, kt_s, p + 1, n)
                        )
                fr = [0]
                fi = [0]

                def maybe_fill(ui, nunits):
                    if fr[0] < len(front):
                        front[fr[0]]()
                        fr[0] += 1
                        return
                    want = (ui + 1) * len(spread) // nunits
                    while fi[0] < min(want, len(spread)):
                        spread[fi[0]]()
                        fi[0] += 1

                for h in (2 * p, 2 * p + 1):
                    mh, ph = h // 2, 64 * (h % 2)
                    units = []
                    for ki in range(mt):
                        for qc in range(128 * ki // 512, nqc):
                            units.append((ki, qc))
                    if p == dt - 1 and h == 2 * p + 1:
                        # last head: phase C becomes the (growing) filler
                        spread = []
                        fi[0] = 0
                    for qc in range(nqc):
                        otp[qc] = otps_tile = psp.tile(
                            [65, 512], F32, name=f"otp{h}_{qc}", tag="ot", bufs=4
                        )
                    pv_pending = None
                    for ui, (ki, qc) in enumerate(units):
                        a = max(128 * ki, 512 * qc)
                        b = 512 * qc + 512
                        st = psp.tile(
                            [128, 512], F32, name=f"st{h}_{ki}_{qc}", tag="st",
                            bufs=2,
                        )
                        nc.tensor.matmul(
                            st[:, 0 : b - a],
                            kt_s[ph : ph + 64, mh, 128 * ki : 128 * ki + 128],
                            qt_s[ph : ph + 64, mh, a:b],
                            start=True,
                            stop=True,
                        )
                        ex = pbe.tile(
                            [128, 512], BF16, name=f"ex{h}_{ki}_{qc}", tag="ex"
                        )
                        nc.scalar.activation(
                            ex[:, 0 : b - a], st[:, 0 : b - a], EXP, scale=scale
                        )
                        if a == 128 * ki:
                            nc.vector.tensor_mul(
                                ex[:, 0:128], ex[:, 0:128], mask_t[:, :]
                            )
                        maybe_fill(ui, len(units))
                        if pv_pending is not None:
                            done_qc = emit_pv(*pv_pending)
                            if (
                                done_qc is not None
                                and p == dt - 1
                                and h == 2 * p + 1
                            ):
                                spread.extend(
                                    (lambda m=m: emit_c(m))
                                    for m in range(4 * done_qc, 4 * done_qc + 4)
                                )
                        pv_pending = (h, ki, qc, a, b, ex)
                    done_qc = emit_pv(*pv_pending)
                    if done_qc is not None and p == dt - 1 and h == 2 * p + 1:
                        spread.extend(
                            (lambda m=m: emit_c(m))
                            for m in range(4 * done_qc, 4 * done_qc + 4)
                        )
                # drain remaining filler for this pair
                while fi[0] < len(spread):
                    spread[fi[0]]()
                    fi[0] += 1

    nc.finalize()
    return nc


_NC_CACHE = {}


def _get_nc(key=(T, 512, 8, KD)):
    if key not in _NC_CACHE:
        _NC_CACHE[key] = build_nc(*key)
    return _NC_CACHE[key]


def make_in_maps(x, Wq, Wk, Wv, Wo, dl=512):
    in_maps = []
    for c in range(NCORES):
        b, g = c // 2, c % 2
        rows = slice(dl * g, dl * (g + 1))
        in_maps.append(
            {
                "xt": np.ascontiguousarray(x[b].T).astype(ml_dtypes.bfloat16),
                "wq": np.ascontiguousarray(Wq[rows, :].T).astype(ml_dtypes.bfloat16),
                "wk": np.ascontiguousarray(Wk[rows, :].T).astype(ml_dtypes.bfloat16),
                "wv": np.ascontiguousarray(Wv[rows, :].T).astype(ml_dtypes.bfloat16),
                "wo": np.ascontiguousarray(Wo[:, rows].T).astype(ml_dtypes.bfloat16),
            }
        )
    return in_maps


def run_spmd(x, Wq, Wk, Wv, Wo, trace=False):
    nc = _get_nc()
    in_maps = make_in_maps(x, Wq, Wk, Wv, Wo)
    res = run_bass_kernel_spmd(nc, in_maps, list(range(NCORES)), trace=trace)
    outs = [res.results[c]["out"] for c in range(NCORES)]
    final = np.stack([outs[2 * b] + outs[2 * b + 1] for b in range(B)])
    return final.astype(np.float32), res


def kernel(x, Wq, Wk, Wv, Wo):
    x = np.asarray(x, dtype=np.float32)
    Wq = np.asarray(Wq, dtype=np.float32)
    Wk = np.asarray(Wk, dtype=np.float32)
    Wv = np.asarray(Wv, dtype=np.float32)
    Wo = np.asarray(Wo, dtype=np.float32)
    out, _ = run_spmd(x, Wq, Wk, Wv, Wo)
    return out


# revision 12
# speedup vs baseline: 1.6186x; 1.1589x over previous
"""Multi-head causal attention (b=4, t=2048, k=1024, h=16) on 8 Trainium2 cores.

Sharding: core c = (batch b=c//2, head-group g=c%2). Each core computes one
batch x 8 heads; the two half-head partial outputs per batch are summed on
host.

Per-core kernel, v5. The tensor engine only sustains full clock under high
duty cycle, and attention is a latency chain (ST matmul -> exp on ACT -> PV
matmul), so:
  - q-major attention: per head, loop q-chunks (512 cols) outer and k-tiles
    inner. Only one PV psum accumulator is live at a time (otp bufs=2),
    freeing psum banks for a depth-4 ST pipeline (st bufs=4) that keeps the
    ACT engine's exp stream saturated.
  - projection and output-projection matmuls are interleaved between
    attention units as PE filler: pair p's attention carries Q/K projections
    of pair p+1; V tiles front-load into pair 0; phase C output projection
    splits into k01 (head-pairs 0,1 - fills head 6) and k23 (qc-gated,
    fills head 7) halves combined by accumulate-add output DMA.
  - normalize: DVE reciprocal_approx_fast on the denominator row (staged
    through SBUF), gpsimd partition_broadcast, DVE multiply; odd heads are
    shifted to partitions 64-127 by sbuf->sbuf DMA.
  PSUM budget: proj(2) + st(4) + otp(2) = 8 banks.
"""
import sys

sys.path.insert(0, "/opt/trn_rl_repo")

import numpy as np
import ml_dtypes

import concourse.bass as bass
import concourse.mybir as mybir
import concourse.tile as tile
from concourse import bacc
from concourse.bass_utils import run_bass_kernel_spmd
from concourse.masks import make_upper_triangular

F32 = mybir.dt.float32
BF16 = mybir.dt.bfloat16
EXP = mybir.ActivationFunctionType.Exp
ADD = mybir.AluOpType.add

B, T, KD, NH, HS = 4, 2048, 1024, 16, 64
NCORES = 8


def build_nc(t=T, dl=512, hl=8, kd=KD):
    nk = kd // 128       # contraction tiles for projections
    mt = t // 128        # t tiles (k-position tiles in attention)
    dt = dl // 128       # local-dim tiles (head pairs)
    nqc = t // 512       # q chunks
    scale = 1.0 / float(np.sqrt(kd))

    nc = bacc.Bacc("TRN2", target_bir_lowering=False, debug=False, num_devices=NCORES)
    xt_d = nc.dram_tensor("xt", [kd, t], BF16, kind="ExternalInput")
    wq_d = nc.dram_tensor("wq", [kd, dl], BF16, kind="ExternalInput")
    wk_d = nc.dram_tensor("wk", [kd, dl], BF16, kind="ExternalInput")
    wv_d = nc.dram_tensor("wv", [kd, dl], BF16, kind="ExternalInput")
    wo_d = nc.dram_tensor("wo", [dl, kd], BF16, kind="ExternalInput")
    out_d = nc.dram_tensor("out", [t, kd], F32, kind="ExternalOutput")

    with tile.TileContext(nc) as tc:
        with (
            tc.tile_pool(name="persist", bufs=1) as pp,
            tc.tile_pool(name="misc", bufs=1) as mp,
            tc.tile_pool(name="pbe", bufs=4) as pbe,
            tc.tile_pool(name="pbm", bufs=4) as pbm,
            tc.tile_pool(name="pco", bufs=3) as pco,
            tc.tile_pool(name="psum", bufs=1, space="PSUM") as psp,
        ):
            qt_s = pp.tile([128, dt, t], BF16)
            kt_s = pp.tile([128, dt, t], BF16)
            v_s = pp.tile([128, mt, hl, 65], BF16)
            ot_s = pp.tile([128, dt, t], BF16)
            xt_s = pp.tile([128, nk, t], BF16)
            wq_s = pp.tile([128, nk, dl], BF16)
            wk_s = pp.tile([128, nk, dl], BF16)
            wv_s = pp.tile([128, nk, dl], BF16)
            wo_s = pp.tile([128, dt, kd], BF16)
            mask_f = mp.tile([128, 128], F32)
            mask_t = mp.tile([128, 128], BF16)
            make_upper_triangular(nc, mask_f[:, :], val=1.0, diag=True)
            nc.vector.tensor_copy(mask_t[:, :], mask_f[:, :])
            nc.vector.memset(v_s[:, :, :, 64], 1.0)

            # --------------- input DMA (fine-grained for fast start) -------
            wq_r = wq_d[:, :].rearrange("(n p) d -> p n d", p=128)
            wk_r = wk_d[:, :].rearrange("(n p) d -> p n d", p=128)
            xt_r = xt_d[:, :].rearrange("(n p) t -> p n t", p=128)
            for k in range(nk):
                nc.scalar.dma_start(wq_s[:, k, :], wq_r[:, k, :])
            for k in range(nk):
                nc.sync.dma_start(xt_s[:, k, 0:512], xt_r[:, k, 0:512])
            for k in range(nk):
                nc.scalar.dma_start(wk_s[:, k, :], wk_r[:, k, :])
            for n in range(1, t // 512):
                nc.sync.dma_start(
                    xt_s[:, :, 512 * n : 512 * n + 512],
                    xt_r[:, :, 512 * n : 512 * n + 512],
                )
            nc.scalar.dma_start(
                wv_s[:, :, :], wv_d[:, :].rearrange("(n p) d -> p n d", p=128)
            )
            nc.scalar.dma_start(
                wo_s[:, :, :], wo_d[:, :].rearrange("(n p) o -> p n o", p=128)
            )

            # --------------- filler emitters -------------------------------
            cnt = [0]

            def emit_qk(w_s, o_s, pair, n, on_act=False):
                cols = slice(512 * n, 512 * n + 512)
                ps = psp.tile([128, 512], F32, name=f"pj{cnt[0]}", tag="proj", bufs=2)
                cnt[0] += 1
                for k in range(nk):
                    nc.tensor.matmul(
                        ps[:, :],
                        w_s[:, k, 128 * pair : 128 * pair + 128],
                        xt_s[:, k, cols],
                        start=(k == 0),
                        stop=(k == nk - 1),
                    )
                if on_act:
                    nc.scalar.copy(o_s[:, pair, cols], ps[:, :])
                else:
                    nc.vector.tensor_copy(o_s[:, pair, cols], ps[:, :])

            def emit_v(m, on_act=False):
                ps = psp.tile([128, 512], F32, name=f"pv{cnt[0]}", tag="proj", bufs=2)
                cnt[0] += 1
                for k in range(nk):
                    nc.tensor.matmul(
                        ps[:, :],
                        xt_s[:, k, 128 * m : 128 * m + 128],
                        wv_s[:, k, :],
                        start=(k == 0),
                        stop=(k == nk - 1),
                    )
                src = ps[:, :].rearrange("p (h d) -> p h d", h=hl)
                if on_act:
                    nc.scalar.copy(v_s[:, m, :, 0:64], src)
                else:
                    nc.vector.tensor_copy(v_s[:, m, :, 0:64], src)

            def emit_c(m, ks, accum):
                """Half of phase C for t-tile m, contracting head-pairs `ks`;
                accumulate into DRAM via DMA add when `accum`."""
                ob = pco.tile([128, kd], F32, name=f"ob{cnt[0]}", tag="ob")
                cnt[0] += 1
                for c in range(kd // 512):
                    ps = psp.tile(
                        [128, 512], F32, name=f"pc{cnt[0]}", tag="proj", bufs=2
                    )
                    cnt[0] += 1
                    for j, k in enumerate(ks):
                        nc.tensor.matmul(
                            ps[:, :],
                            ot_s[:, k, 128 * m : 128 * m + 128],
                            wo_s[:, k, 512 * c : 512 * c + 512],
                            start=(j == 0),
                            stop=(j == len(ks) - 1),
                        )
                    nc.vector.tensor_copy(ob[:, 512 * c : 512 * c + 512], ps[:, :])
                if accum:
                    nc.gpsimd.dma_start(
                        out_d[128 * m : 128 * m + 128, :], ob[:, :], accum_op=ADD
                    )
                else:
                    nc.sync.dma_start(out_d[128 * m : 128 * m + 128, :], ob[:, :])

            # --------------- prologue: QK(pair0) + V(0..3) -----------------
            for n in range(4):
                emit_qk(wq_s, qt_s, 0, n, on_act=True)
                emit_qk(wk_s, kt_s, 0, n, on_act=True)
            for m in range(4):
                emit_v(m, on_act=True)

            # --------------- fused attention + filler ----------------------
            def emit_pv(h, ki, qc, a, b, ex, otp):
                mh, ph = h // 2, 64 * (h % 2)
                nc.tensor.matmul(
                    otp[0:65, a - 512 * qc : b - 512 * qc],
                    v_s[:, ki, h, :],
                    ex[:, 0 : b - a],
                    start=(ki == 0),
                    stop=(ki == 4 * qc + 3),
                )
                if ki != 4 * qc + 3:
                    return False
                den = pbm.tile([1, 512], F32, name=f"dn{h}_{qc}", tag="den")
                nc.vector.tensor_copy(den[:, :], otp[64:65, :])
                rec = pbm.tile([1, 512], F32, name=f"rc{h}_{qc}", tag="rec")
                nc.vector.reciprocal_approx_fast(rec[:, :], den[:, :])
                bc = pbm.tile([64, 512], F32, name=f"bc{h}_{qc}", tag="bc")
                nc.gpsimd.partition_broadcast(bc[:, :], rec[0:1, :])
                cols = slice(512 * qc, 512 * qc + 512)
                if ph == 0:
                    nc.vector.tensor_mul(ot_s[0:64, mh, cols], otp[0:64, :], bc[:, :])
                else:
                    sc = pbm.tile([64, 512], BF16, name=f"sc{h}_{qc}", tag="sc")
                    nc.vector.tensor_mul(sc[:, :], otp[0:64, :], bc[:, :])
                    nc.sync.dma_start(ot_s[64:128, mh, cols], sc[:, :])
                return True

            for p in range(dt):
                front = []
                spread = []
                if p == 0:
                    front += [(lambda m=m: emit_v(m)) for m in range(4, mt)]
                if p < dt - 1:
                    for n in range(4):
                        spread.append(lambda n=n, p=p: emit_qk(wq_s, qt_s, p + 1, n))
                        spread.append(lambda n=n, p=p: emit_qk(wk_s, kt_s, p + 1, n))
                if p == dt - 1:
                    # head 6's filler: phase C halves over head-pairs 0,1
                    spread += [(lambda m=m: emit_c(m, (0, 1), False)) for m in range(mt)]
                fr = [0]
                fi = [0]
                pui = [0]
                nunits_head = sum(4 * qc + 4 for qc in range(nqc))
                # last pair paces per-head (h6: C01, h7: qc-gated C23)
                nunits_pair = nunits_head if p == dt - 1 else 2 * nunits_head

                def maybe_fill():
                    pui[0] += 1
                    if fr[0] < len(front):
                        front[fr[0]]()
                        fr[0] += 1
                        return
                    want = pui[0] * len(spread) // nunits_pair
                    while fi[0] < min(want, len(spread)):
                        spread[fi[0]]()
                        fi[0] += 1

                for h in (2 * p, 2 * p + 1):
                    mh, ph = h // 2, 64 * (h % 2)
                    if p == dt - 1 and h == 2 * p + 1:
                        # drain h6's leftovers, then h7 runs qc-gated C(2,3)
                        while fi[0] < len(spread):
                            spread[fi[0]]()
                            fi[0] += 1
                        spread = []
                        fi[0] = 0
                        pui[0] = 0
                    pv_pending = None
                    for qc in range(nqc):
                        otp = psp.tile(
                            [65, 512], F32, name=f"otp{h}_{qc}", tag="ot", bufs=2
                        )
                        for ki in range(4 * qc + 4):
                            a = max(128 * ki, 512 * qc)
                            b = 512 * qc + 512
                            st = psp.tile(
                                [128, 512], F32, name=f"st{h}_{ki}_{qc}",
                                tag="st", bufs=4,
                            )
                            nc.tensor.matmul(
                                st[:, 0 : b - a],
                                kt_s[ph : ph + 64, mh, 128 * ki : 128 * ki + 128],
                                qt_s[ph : ph + 64, mh, a:b],
                                start=True,
                                stop=True,
                            )
                            ex = pbe.tile(
                                [128, 512], BF16, name=f"ex{h}_{ki}_{qc}", tag="ex"
                            )
                            nc.scalar.activation(
                                ex[:, 0 : b - a], st[:, 0 : b - a], EXP, scale=scale
                            )
                            if a == 128 * ki:
                                nc.vector.tensor_mul(
                                    ex[:, 0:128], ex[:, 0:128], mask_t[:, :]
                                )
                            maybe_fill()
                            if pv_pending is not None:
                                done = emit_pv(*pv_pending)
                                if done and p == dt - 1 and h == 2 * p + 1:
                                    dqc = pv_pending[2]
                                    spread.extend(
                                        (lambda m=m: emit_c(m, (2, 3), True))
                                        for m in range(4 * dqc, 4 * dqc + 4)
                                    )
                            pv_pending = (h, ki, qc, a, b, ex, otp)
                    done = emit_pv(*pv_pending)
                    if done and p == dt - 1 and h == 2 * p + 1:
                        dqc = pv_pending[2]
                        spread.extend(
                            (lambda m=m: emit_c(m, (2, 3), True))
                            for m in range(4 * dqc, 4 * dqc + 4)
                        )
                # drain remaining filler for this pair
                while fi[0] < len(spread):
                    spread[fi[0]]()
                    fi[0] += 1

    nc.finalize()
    return nc


_NC_CACHE = {}


def _get_nc(key=(T, 512, 8, KD)):
    if key not in _NC_CACHE:
        _NC_CACHE[key] = build_nc(*key)
    return _NC_CACHE[key]


def make_in_maps(x, Wq, Wk, Wv, Wo, dl=512):
    in_maps = []
    for c in range(NCORES):
        b, g = c // 2, c % 2
        rows = slice(dl * g, dl * (g + 1))
        in_maps.append(
            {
                "xt": np.ascontiguousarray(x[b].T).astype(ml_dtypes.bfloat16),
                "wq": np.ascontiguousarray(Wq[rows, :].T).astype(ml_dtypes.bfloat16),
                "wk": np.ascontiguousarray(Wk[rows, :].T).astype(ml_dtypes.bfloat16),
                "wv": np.ascontiguousarray(Wv[rows, :].T).astype(ml_dtypes.bfloat16),
                "wo": np.ascontiguousarray(Wo[:, rows].T).astype(ml_dtypes.bfloat16),
            }
        )
    return in_maps


def run_spmd(x, Wq, Wk, Wv, Wo, trace=False):
    nc = _get_nc()
    in_maps = make_in_maps(x, Wq, Wk, Wv, Wo)
    res = run_bass_kernel_spmd(nc, in_maps, list(range(NCORES)), trace=trace)
    outs = [res.results[c]["out"] for c in range(NCORES)]
    final = np.stack([outs[2 * b] + outs[2 * b + 1] for b in range(B)])
    return final.astype(np.float32), res


def kernel(x, Wq, Wk, Wv, Wo):
    x = np.asarray(x, dtype=np.float32)
    Wq = np.asarray(Wq, dtype=np.float32)
    Wk = np.asarray(Wk, dtype=np.float32)
    Wv = np.asarray(Wv, dtype=np.float32)
    Wo = np.asarray(Wo, dtype=np.float32)
    out, _ = run_spmd(x, Wq, Wk, Wv, Wo)
    return out


# revision 14
# speedup vs baseline: 1.7739x; 1.0959x over previous
"""Multi-head causal attention (b=4, t=2048, k=1024, h=16) on 8 Trainium2 cores.

Sharding: core c = (batch b=c//2, head-group g=c%2). Each core computes one
batch x 8 heads; partial outputs (half heads each, and a further k01/k23
output-projection split) are summed on host.

Per-core kernel, v6. The tensor engine only sustains full clock under high
duty cycle and attention is a latency chain (ST -> exp on ACT -> PV), so:
  - q-major attention; consecutive k-tile pairs share one 2-bank psum tile
    so ONE exp covers both (20 exp calls/head instead of 40).
  - projection / output-projection matmuls interleave between attention
    groups as PE filler. Only the first Q/K chunk runs as prologue; V tiles
    and remaining Q/K chunks front-load into pair 0's groups in a
    dependency-safe order. Phase C splits into k01 (fills head 6) and k23
    (qc-gated, fills head 7) halves written to separate DRAM outputs that
    the host adds.
  - normalize: DVE reciprocal_approx_fast (denominator staged via SBUF),
    gpsimd partition_broadcast, DVE multiply; odd heads shift to
    partitions 64-127 via sbuf->sbuf DMA.
  PSUM: proj(2) + st(2x2-bank=4) + otp(2) = 8 banks.
"""
import sys

sys.path.insert(0, "/opt/trn_rl_repo")

import numpy as np
import ml_dtypes

import concourse.bass as bass
import concourse.mybir as mybir
import concourse.tile as tile
from concourse import bacc
from concourse.bass_utils import run_bass_kernel_spmd
from concourse.masks import make_upper_triangular

F32 = mybir.dt.float32
BF16 = mybir.dt.bfloat16
EXP = mybir.ActivationFunctionType.Exp

B, T, KD, NH, HS = 4, 2048, 1024, 16, 64
NCORES = 8


def build_nc(t=T, dl=512, hl=8, kd=KD):
    nk = kd // 128       # contraction tiles for projections
    mt = t // 128        # t tiles (k-position tiles in attention)
    dt = dl // 128       # local-dim tiles (head pairs)
    nqc = t // 512       # q chunks
    scale = 1.0 / float(np.sqrt(kd))

    nc = bacc.Bacc("TRN2", target_bir_lowering=False, debug=False, num_devices=NCORES)
    xt_d = nc.dram_tensor("xt", [kd, t], BF16, kind="ExternalInput")
    wq_d = nc.dram_tensor("wq", [kd, dl], BF16, kind="ExternalInput")
    wk_d = nc.dram_tensor("wk", [kd, dl], BF16, kind="ExternalInput")
    wv_d = nc.dram_tensor("wv", [kd, dl], BF16, kind="ExternalInput")
    wo_d = nc.dram_tensor("wo", [dl, kd], BF16, kind="ExternalInput")
    out_d = nc.dram_tensor("out", [t, kd], F32, kind="ExternalOutput")
    out2_d = nc.dram_tensor("out2", [t, kd], F32, kind="ExternalOutput")

    with tile.TileContext(nc) as tc:
        with (
            tc.tile_pool(name="persist", bufs=1) as pp,
            tc.tile_pool(name="misc", bufs=1) as mp,
            tc.tile_pool(name="pbe", bufs=4) as pbe,
            tc.tile_pool(name="pbm", bufs=4) as pbm,
            tc.tile_pool(name="pco", bufs=3) as pco,
            tc.tile_pool(name="psum", bufs=1, space="PSUM") as psp,
        ):
            qt_s = pp.tile([128, dt, t], BF16)
            kt_s = pp.tile([128, dt, t], BF16)
            v_s = pp.tile([128, mt, hl, 65], BF16)
            ot_s = pp.tile([128, dt, t], BF16)
            xt_s = pp.tile([128, nk, t], BF16)
            wq_s = pp.tile([128, nk, dl], BF16)
            wk_s = pp.tile([128, nk, dl], BF16)
            wv_s = pp.tile([128, nk, dl], BF16)
            wo_s = pp.tile([128, dt, kd], BF16)
            mask_f = mp.tile([128, 128], F32)
            mask_t = mp.tile([128, 128], BF16)
            make_upper_triangular(nc, mask_f[:, :], val=1.0, diag=True)
            nc.vector.tensor_copy(mask_t[:, :], mask_f[:, :])
            nc.vector.memset(v_s[:, :, :, 64], 1.0)

            # --------------- input DMA (fine-grained for fast start) -------
            wq_r = wq_d[:, :].rearrange("(n p) d -> p n d", p=128)
            wk_r = wk_d[:, :].rearrange("(n p) d -> p n d", p=128)
            wv_r = wv_d[:, :].rearrange("(n p) d -> p n d", p=128)
            xt_r = xt_d[:, :].rearrange("(n p) t -> p n t", p=128)
            for k in range(nk):
                nc.scalar.dma_start(wq_s[:, k, :], wq_r[:, k, :])
            for k in range(nk):
                nc.sync.dma_start(xt_s[:, k, 0:512], xt_r[:, k, 0:512])
            for k in range(nk):
                nc.scalar.dma_start(wk_s[:, k, :], wk_r[:, k, :])
            for k in range(nk):
                nc.scalar.dma_start(wv_s[:, k, :], wv_r[:, k, :])
            for n in range(1, t // 512):
                nc.sync.dma_start(
                    xt_s[:, :, 512 * n : 512 * n + 512],
                    xt_r[:, :, 512 * n : 512 * n + 512],
                )
            nc.scalar.dma_start(
                wo_s[:, :, :], wo_d[:, :].rearrange("(n p) o -> p n o", p=128)
            )

            # --------------- filler emitters -------------------------------
            cnt = [0]

            def emit_qk(w_s, o_s, pair, n, on_act=False):
                cols = slice(512 * n, 512 * n + 512)
                ps = psp.tile([128, 512], F32, name=f"pj{cnt[0]}", tag="proj", bufs=2)
                cnt[0] += 1
                for k in range(nk):
                    nc.tensor.matmul(
                        ps[:, :],
                        w_s[:, k, 128 * pair : 128 * pair + 128],
                        xt_s[:, k, cols],
                        start=(k == 0),
                        stop=(k == nk - 1),
                    )
                if on_act:
                    nc.scalar.copy(o_s[:, pair, cols], ps[:, :])
                else:
                    nc.vector.tensor_copy(o_s[:, pair, cols], ps[:, :])

            def emit_v(m, on_act=False):
                ps = psp.tile([128, 512], F32, name=f"pv{cnt[0]}", tag="proj", bufs=2)
                cnt[0] += 1
                for k in range(nk):
                    nc.tensor.matmul(
                        ps[:, :],
                        xt_s[:, k, 128 * m : 128 * m + 128],
                        wv_s[:, k, :],
                        start=(k == 0),
                        stop=(k == nk - 1),
                    )
                src = ps[:, :].rearrange("p (h d) -> p h d", h=hl)
                if on_act:
                    nc.scalar.copy(v_s[:, m, :, 0:64], src)
                else:
                    nc.vector.tensor_copy(v_s[:, m, :, 0:64], src)

            def emit_c(m, ks, dst_d):
                """Half of phase C for t-tile m, contracting head-pairs `ks`."""
                ob = pco.tile([128, kd], F32, name=f"ob{cnt[0]}", tag="ob")
                cnt[0] += 1
                for c in range(kd // 512):
                    ps = psp.tile(
                        [128, 512], F32, name=f"pc{cnt[0]}", tag="proj", bufs=2
                    )
                    cnt[0] += 1
                    for j, k in enumerate(ks):
                        nc.tensor.matmul(
                            ps[:, :],
                            ot_s[:, k, 128 * m : 128 * m + 128],
                            wo_s[:, k, 512 * c : 512 * c + 512],
                            start=(j == 0),
                            stop=(j == len(ks) - 1),
                        )
                    nc.vector.tensor_copy(ob[:, 512 * c : 512 * c + 512], ps[:, :])
                nc.sync.dma_start(dst_d[128 * m : 128 * m + 128, :], ob[:, :])

            # --------------- prologue: first Q/K chunk only ----------------
            emit_qk(wq_s, qt_s, 0, 0, on_act=True)
            emit_qk(wk_s, kt_s, 0, 0, on_act=True)

            # --------------- fused attention + filler ----------------------
            def emit_pv(h, ki, qc, a, b, ex, exo, otp):
                """PV for one (ki, qc) unit; ex columns [exo, exo + b - a)."""
                nc.tensor.matmul(
                    otp[0:65, a - 512 * qc : b - 512 * qc],
                    v_s[:, ki, h, :],
                    ex[:, exo : exo + b - a],
                    start=(ki == 0),
                    stop=(ki == 4 * qc + 3),
                )
                if ki != 4 * qc + 3:
                    return False
                mh, ph = h // 2, 64 * (h % 2)
                den = pbm.tile([1, 512], F32, name=f"dn{h}_{qc}", tag="den")
                nc.vector.tensor_copy(den[:, :], otp[64:65, :])
                rec = pbm.tile([1, 512], F32, name=f"rc{h}_{qc}", tag="rec")
                nc.vector.reciprocal_approx_fast(rec[:, :], den[:, :])
                bc = pbm.tile([64, 512], F32, name=f"bc{h}_{qc}", tag="bc")
                nc.gpsimd.partition_broadcast(bc[:, :], rec[0:1, :])
                cols = slice(512 * qc, 512 * qc + 512)
                if ph == 0:
                    nc.vector.tensor_mul(ot_s[0:64, mh, cols], otp[0:64, :], bc[:, :])
                else:
                    sc = pbm.tile([64, 512], BF16, name=f"sc{h}_{qc}", tag="sc")
                    nc.vector.tensor_mul(sc[:, :], otp[0:64, :], bc[:, :])
                    nc.sync.dma_start(ot_s[64:128, mh, cols], sc[:, :])
                return True

            ngrp_head = sum((4 * qc + 4) // 2 for qc in range(nqc))  # 20

            for p in range(dt):
                front = []
                spread = []
                if p == 0:
                    # dependency-safe front-load order (2 items per group):
                    # QK chunk n must precede attention q-chunk n; V tiles
                    # stream ahead of their PV consumers.
                    front = [
                        lambda: emit_v(0),
                        lambda: emit_v(1),
                        lambda: emit_qk(wq_s, qt_s, 0, 1),
                        lambda: emit_qk(wk_s, kt_s, 0, 1),
                        lambda: emit_v(2),
                        lambda: emit_v(3),
                        lambda: emit_v(4),
                        lambda: emit_qk(wq_s, qt_s, 0, 2),
                        lambda: emit_qk(wk_s, kt_s, 0, 2),
                        lambda: emit_v(5),
                        lambda: emit_v(6),
                        lambda: emit_v(7),
                        lambda: emit_qk(wq_s, qt_s, 0, 3),
                        lambda: emit_qk(wk_s, kt_s, 0, 3),
                    ] + [(lambda m=m: emit_v(m)) for m in range(8, mt)]
                if p < dt - 1:
                    for n in range(4):
                        spread.append(lambda n=n, p=p: emit_qk(wq_s, qt_s, p + 1, n))
                        spread.append(lambda n=n, p=p: emit_qk(wk_s, kt_s, p + 1, n))
                if p == dt - 1:
                    spread += [
                        (lambda m=m: emit_c(m, (0, 1), out2_d)) for m in range(mt)
                    ]
                fr = [0]
                fi = [0]
                pui = [0]
                npace = ngrp_head if p == dt - 1 else 2 * ngrp_head

                def maybe_fill():
                    pui[0] += 1
                    took = 0
                    while fr[0] < len(front) and took < 2:
                        front[fr[0]]()
                        fr[0] += 1
                        took += 1
                    if took:
                        return
                    want = pui[0] * len(spread) // npace
                    while fi[0] < min(want, len(spread)):
                        spread[fi[0]]()
                        fi[0] += 1

                for h in (2 * p, 2 * p + 1):
                    mh, ph = h // 2, 64 * (h % 2)
                    if p == dt - 1 and h == 2 * p + 1:
                        while fi[0] < len(spread):
                            spread[fi[0]]()
                            fi[0] += 1
                        spread = []
                        fi[0] = 0
                        pui[0] = 0
                    pv_pending = []
                    for qc in range(nqc):
                        otp = psp.tile(
                            [65, 512], F32, name=f"otp{h}_{qc}", tag="ot", bufs=2
                        )
                        for ki0 in range(0, 4 * qc + 4, 2):
                            st = psp.tile(
                                [128, 1024], F32, name=f"st{h}_{ki0}_{qc}",
                                tag="st", bufs=2,
                            )
                            ex = pbe.tile(
                                [128, 1024], BF16, name=f"ex{h}_{ki0}_{qc}",
                                tag="ex",
                            )
                            # place the two units contiguously (no unwritten
                            # psum gap for exp): unit 2 at w1 if both fit in
                            # bank 0, else at the bank-1 boundary
                            b = 512 * qc + 512
                            a1 = max(128 * ki0, 512 * qc)
                            a2 = max(128 * (ki0 + 1), 512 * qc)
                            w1, w2 = b - a1, b - a2
                            o2 = w1 if w1 + w2 <= 512 else 512
                            ws = [(ki0, a1, 0), (ki0 + 1, a2, o2)]
                            for ki, a, off in ws:
                                nc.tensor.matmul(
                                    st[:, off : off + b - a],
                                    kt_s[ph : ph + 64, mh, 128 * ki : 128 * ki + 128],
                                    qt_s[ph : ph + 64, mh, a:b],
                                    start=True,
                                    stop=True,
                                )
                            nc.scalar.activation(
                                ex[:, 0 : o2 + w2], st[:, 0 : o2 + w2],
                                EXP, scale=scale,
                            )
                            for ki, a, off in ws:
                                if a == 128 * ki:
                                    nc.vector.tensor_mul(
                                        ex[:, off : off + 128],
                                        ex[:, off : off + 128],
                                        mask_t[:, :],
                                    )
                            maybe_fill()
                            for args in pv_pending:
                                done = emit_pv(*args)
                                if done and p == dt - 1 and h == 2 * p + 1:
                                    dqc = args[2]
                                    spread.extend(
                                        (lambda m=m: emit_c(m, (2, 3), out_d))
                                        for m in range(4 * dqc, 4 * dqc + 4)
                                    )
                            pv_pending = [
                                (h, ki, qc, a, b, ex, off, otp)
                                for ki, a, off in ws
                            ]
                    for args in pv_pending:
                        done = emit_pv(*args)
                        if done and p == dt - 1 and h == 2 * p + 1:
                            dqc = args[2]
                            spread.extend(
                                (lambda m=m: emit_c(m, (2, 3), out_d))
                                for m in range(4 * dqc, 4 * dqc + 4)
                            )
                while fi[0] < len(spread):
                    spread[fi[0]]()
                    fi[0] += 1

    nc.finalize()
    return nc


_NC_CACHE = {}


def _get_nc(key=(T, 512, 8, KD)):
    if key not in _NC_CACHE:
        _NC_CACHE[key] = build_nc(*key)
    return _NC_CACHE[key]


def make_in_maps(x, Wq, Wk, Wv, Wo, dl=512):
    in_maps = []
    for c in range(NCORES):
        b, g = c // 2, c % 2
        rows = slice(dl * g, dl * (g + 1))
        in_maps.append(
            {
                "xt": np.ascontiguousarray(x[b].T).astype(ml_dtypes.bfloat16),
                "wq": np.ascontiguousarray(Wq[rows, :].T).astype(ml_dtypes.bfloat16),
                "wk": np.ascontiguousarray(Wk[rows, :].T).astype(ml_dtypes.bfloat16),
                "wv": np.ascontiguousarray(Wv[rows, :].T).astype(ml_dtypes.bfloat16),
                "wo": np.ascontiguousarray(Wo[:, rows].T).astype(ml_dtypes.bfloat16),
            }
        )
    return in_maps


def run_spmd(x, Wq, Wk, Wv, Wo, trace=False):
    nc = _get_nc()
    in_maps = make_in_maps(x, Wq, Wk, Wv, Wo)
    res = run_bass_kernel_spmd(nc, in_maps, list(range(NCORES)), trace=trace)
    outs = [
        res.results[c]["out"] + res.results[c]["out2"] for c in range(NCORES)
    ]
    final = np.stack([outs[2 * b] + outs[2 * b + 1] for b in range(B)])
    return final.astype(np.float32), res


def kernel(x, Wq, Wk, Wv, Wo):
    x = np.asarray(x, dtype=np.float32)
    Wq = np.asarray(Wq, dtype=np.float32)
    Wk = np.asarray(Wk, dtype=np.float32)
    Wv = np.asarray(Wv, dtype=np.float32)
    Wo = np.asarray(Wo, dtype=np.float32)
    out, _ = run_spmd(x, Wq, Wk, Wv, Wo)
    return out
